# revision 12
# baseline (speedup 1.0000x reference)
"""MultiHeadContrastive loss on 8 TRN2 NeuronCores (Bass/Tile SPMD).

Strategy: data-parallel over the anchor (row) dimension; rows are
permuted host-side so all background rows (label==0) come first.  Each
core owns N/8 = 1024 rows: runs the two projection MLPs for its rows
(fg head first), normalizes, transposes to [D, rows], AllGathers z in
two pieces (fg first, so the fg j-loop overlaps the cls AllGather),
AllReduces the per-class embedding sums, then computes its rows'
contributions to both losses.

The sim work runs in [own-i on partitions, all-j on free] layout:
sim tiles are PE matmuls (lhsT = own z.T chunk, rhs = gathered z.T),
and the exp row-sums come for free from the scalar engine's fused
activation accumulator (activation(Exp, accum_out=...)) - no exp
values are kept and no PE reduction matmuls are needed.  The
fg-positive (masked) numerator is total minus the sum over the first
n_bg columns, which is one small extra activation per i-chunk thanks
to the bg-first permutation.

Supcon positive-pair sums use linearity: sum_{j in class c} z_i.z_j =
z_i . zbar_c, with zbar (and the class histogram) computed once via a
one-hot matmul + AllReduce.
"""
import numpy as np
import ml_dtypes

import concourse.bacc as bacc
import concourse.mybir as mybir
import concourse.tile as tile
import concourse.bass_utils as bass_utils
from concourse.tile_rust import add_dep_helper

NCORES = 8
N, C, H, DF, DC = 8192, 1024, 256, 64, 128
SH = N // NCORES      # 1024 rows per core
NIC = SH // 128       # 8 i-chunks of 128 rows
NCLS = 21
EPS = 1e-8
TAU = 0.2

JT = 2048             # j-tile width (free dim of one sim psum tile)
NJT = N // JT         # 4 j-tiles

BF16 = mybir.dt.bfloat16
F32 = mybir.dt.float32
AF = mybir.ActivationFunctionType
ALU = mybir.AluOpType

_cached = {}


def _build(n_bg):
    assert 0 <= n_bg <= JT
    bgw = max(512, ((n_bg + 511) // 512) * 512) if n_bg > 0 else 0

    nc = bacc.Bacc("TRN2", target_bir_lowering=False, debug=False,
                   num_devices=NCORES)

    def inp(name, shape, dt):
        return nc.dram_tensor(name, shape, dt, kind="ExternalInput")

    xT = inp("xT", [C, SH], BF16)            # own rows, transposed
    w1f = inp("w1f", [C, H], BF16)
    w1c = inp("w1c", [C, H], BF16)
    b1f = inp("b1f", [128, H // 128], F32)   # partition-major
    b1c = inp("b1c", [128, H // 128], F32)
    w2f = inp("w2f", [H, DF], BF16)
    w2c = inp("w2c", [H, DC], BF16)
    b2f8 = inp("b2f8", [128, NIC * DF], F32)   # b2 bcast, tiled per i-chunk
    b2c8 = inp("b2c8", [128, NIC * DC], F32)
    fgown = inp("fgown", [128, NIC], F32)    # own fg mask
    iou = inp("iou", [128, NIC], F32)        # own ious
    ohb = inp("ohb", [128, NIC * NCLS], BF16)  # own-label one-hot per i-chunk
    ident = inp("ident", [128, 128], BF16)
    identF = inp("identF", [128, 128], F32)

    psums = nc.dram_tensor("psums", [1, 8], F32, kind="ExternalOutput")

    # collective buffers
    zpackF = nc.dram_tensor("zpackF", [DF, SH], BF16)
    zgathF = nc.dram_tensor("zgathF", [NCORES * DF, SH], BF16,
                            addr_space="Shared")
    zpackC = nc.dram_tensor("zpackC", [DC, SH], BF16)
    zgathC = nc.dram_tensor("zgathC", [NCORES * DC, SH], BF16,
                            addr_space="Shared")
    cbL = nc.dram_tensor("cbL", [NCLS, DC + 1], F32)
    cbR = nc.dram_tensor("cbR", [NCLS, DC + 1], F32, addr_space="Shared")
    dumL = nc.dram_tensor("dumL", [1, 4], F32)
    dumR = nc.dram_tensor("dumR", [1, 4], F32, addr_space="Shared")

    rg = [list(range(NCORES))]

    with tile.TileContext(nc) as tc:
        with (
            tc.tile_pool(name="persist", bufs=1) as P,
            tc.tile_pool(name="work", bufs=2) as W,
        ):
            # dummy collective: absorbs the one-time cross-core rendezvous
            # barrier while the MLP runs, so the real AllGather isn't
            # delayed behind it
            dum_inst = nc.gpsimd.collective_compute(
                "AllReduce", ALU.add, replica_groups=rg,
                ins=[dumL.ap().opt()], outs=[dumR.ap().opt()])

            # ---- load persistent inputs into SBUF ----
            xT_sb = P.tile([128, (C // 128) * SH], BF16, tag="xT")
            xT_r = xT.ap().rearrange("(c p) r -> p c r", p=128)
            w1f_sb = P.tile([128, (C // 128) * H], BF16, tag="w1f")
            w1f_r = w1f.ap().rearrange("(c p) h -> p c h", p=128)
            w1c_sb = P.tile([128, (C // 128) * H], BF16, tag="w1c")
            w1c_r = w1c.ap().rearrange("(c p) h -> p c h", p=128)
            for c in range(C // 128):
                nc.sync.dma_start(w1f_sb[:, c * H:(c + 1) * H],
                                  w1f_r[:, c:c + 1, :])
                nc.sync.dma_start(xT_sb[:, c * SH:(c + 1) * SH],
                                  xT_r[:, c:c + 1, :])
            b1f_sb = P.tile([128, H // 128], F32, tag="b1f")
            nc.sync.dma_start(b1f_sb[:, :], b1f.ap())
            w2f_sb = P.tile([128, (H // 128) * DF], BF16, tag="w2f")
            nc.sync.dma_start(w2f_sb[:, :], w2f.ap().rearrange(
                "(m p) d -> p m d", p=128))
            b2f8_sb = P.tile([128, NIC * DF], F32, tag="b2f8")
            nc.sync.dma_start(b2f8_sb[:, :], b2f8.ap())
            ident_sb = P.tile([128, 128], BF16, tag="ident")
            nc.sync.dma_start(ident_sb[:, :], ident.ap())
            # cls-side inputs (queued after the fg-critical ones)
            for c in range(C // 128):
                nc.sync.dma_start(w1c_sb[:, c * H:(c + 1) * H],
                                  w1c_r[:, c:c + 1, :])
            b1c_sb = P.tile([128, H // 128], F32, tag="b1c")
            nc.sync.dma_start(b1c_sb[:, :], b1c.ap())
            w2c_sb = P.tile([128, (H // 128) * DC], BF16, tag="w2c")
            nc.sync.dma_start(w2c_sb[:, :], w2c.ap().rearrange(
                "(m p) d -> p m d", p=128))
            b2c8_sb = P.tile([128, NIC * DC], F32, tag="b2c8")
            nc.sync.dma_start(b2c8_sb[:, :], b2c8.ap())
            fgown_sb = P.tile([128, NIC], F32, tag="fgown")
            nc.sync.dma_start(fgown_sb[:, :], fgown.ap())
            iou_sb = P.tile([128, NIC], F32, tag="iou")
            nc.sync.dma_start(iou_sb[:, :], iou.ap())
            ohb_sb = P.tile([128, NIC * NCLS], BF16, tag="ohb")
            nc.sync.dma_start(ohb_sb[:, :], ohb.ap())
            identF_sb = P.tile([128, 128], F32, tag="identF")
            nc.sync.dma_start(identF_sb[:, :], identF.ap())

            onesR_sb = P.tile([1, 128], F32, tag="onesR")    # outer-product lhsT
            nc.vector.memset(onesR_sb[:, :], 1.0)
            onesP_sb = P.tile([128, 1], F32, tag="onesP")    # final reduce lhsT
            nc.vector.memset(onesP_sb[:, :], 1.0)
            eps2_sb = P.tile([128, 1], F32, tag="eps2")
            nc.vector.memset(eps2_sb[:, :], 2.0 * EPS)
            eps1_sb = P.tile([128, 1], F32, tag="eps1")
            nc.vector.memset(eps1_sb[:, :], EPS)

            # persistent SBUF tiles
            znfT_sb = P.tile([64, SH], BF16, tag="znfT")
            zncT_sb = P.tile([128, SH], BF16, tag="zncT")
            ssqf_sb = P.tile([128, NIC], F32, tag="ssqf")
            ssqc_sb = P.tile([128, NIC], F32, tag="ssqc")
            spos_sb = P.tile([128, NIC], F32, tag="spos")
            npos_sb = P.tile([128, NIC], F32, tag="npos")
            zfT_all = P.tile([64, N], BF16, tag="zfT_all")
            zcT_all = P.tile([128, N], BF16, tag="zcT_all")
            cb_sb = P.tile([NCLS, DC + 1], F32, tag="cb_sb")
            cbl_sb = P.tile([NCLS, DC + 1], F32, tag="cbl_sb")
            zbcT_sb = P.tile([128, NCLS], BF16, tag="zbcT_sb")
            hist_sb = P.tile([1, NCLS], F32, tag="hist_sb")
            fgtot_sb = P.tile([1, 1], F32, tag="fgtot")
            histB_sb = P.tile([128, NCLS], F32, tag="histB")
            ftB_sb = P.tile([128, 1], F32, tag="ftB")
            # exp row-sum accumulators: per i-chunk 8 slots
            # fg: slots 0..NJT-1 = j-tiles, slot NJT = bg prefix
            accF = P.tile([128, NIC * 8], F32, tag="accF")
            accC = P.tile([128, NIC * 8], F32, tag="accC")

            # ================= fg head pipeline =================
            hTf_sb = P.tile([128, (H // 128) * SH], BF16, tag="hTf")
            PH1ctx = tc.tile_pool(name="ph1", bufs=1, space="PSUM")
            PH1 = PH1ctx.__enter__()
            for m in range(H // 128):           # 2 H-chunks
                pq = [PH1.tile([128, 256], F32, tag=f"hps{q}",
                               name=f"hps{q}", bufs=(2 if q < 3 else 1))
                      for q in range(4)]
                for c in range(C // 128):       # 8 K-chunks
                    for q in range(4):
                        nc.tensor.matmul(
                            pq[q][:, :],
                            lhsT=w1f_sb[:, c * H + m * 128:c * H + (m + 1) * 128],
                            rhs=xT_sb[:, c * SH + q * 256:c * SH + q * 256 + 256],
                            start=(c == 0), stop=(c == C // 128 - 1))
                for q in range(4):
                    nc.vector.tensor_scalar(
                        hTf_sb[:, m * SH + q * 256:m * SH + q * 256 + 256],
                        pq[q][:, :], b1f_sb[:, m:m + 1], 0.0,
                        ALU.add, ALU.max)
            PH1ctx.__exit__(None, None, None)

            PZctx = tc.tile_pool(name="pzf", bufs=1, space="PSUM")
            PZ = PZctx.__enter__()
            PTctx = tc.tile_pool(name="ptrf", bufs=1, space="PSUM")
            PT = PTctx.__enter__()
            zf_ps = PZ.tile([128, NIC * DF], F32, tag="zf")      # 1 bank
            for ic in range(NIC):
                for hm in range(H // 128):
                    nc.tensor.matmul(
                        zf_ps[:, ic * DF:(ic + 1) * DF],
                        lhsT=hTf_sb[:, hm * SH + ic * 128:hm * SH + ic * 128 + 128],
                        rhs=w2f_sb[:, hm * DF:(hm + 1) * DF],
                        start=(hm == 0), stop=(hm == H // 128 - 1))
            ztf = P.tile([128, NIC * DF], F32, tag="ztf")
            nc.vector.tensor_add(ztf[:, :], zf_ps[:, :], b2f8_sb[:, :])
            sqf = W.tile([128, NIC * DF], F32, tag="sqf")
            nc.vector.tensor_mul(sqf[:, :], ztf[:, :], ztf[:, :])
            sqf_v = sqf[:, :].rearrange("p (i c) -> p i c", i=NIC)
            n2f = P.tile([128, NIC], F32, tag="n2f")
            nc.vector.tensor_reduce(n2f[:, :], sqf_v,
                                    mybir.AxisListType.X, ALU.add)
            lnf = P.tile([128, NIC], F32, tag="lnf")
            nc.scalar.activation(lnf[:, :], n2f[:, :], AF.Ln)
            ninvf = P.tile([128, NIC], F32, tag="ninvf")
            nc.scalar.activation(ninvf[:, :], lnf[:, :], AF.Exp, scale=-0.5)
            znf = P.tile([128, NIC * DF], BF16, tag="znf")
            for ic in range(NIC):
                nc.vector.tensor_scalar_mul(
                    znf[:, ic * DF:(ic + 1) * DF],
                    ztf[:, ic * DF:(ic + 1) * DF], ninvf[:, ic:ic + 1])
            # ssq of the bf16-rounded zn
            sqzf = W.tile([128, NIC * DF], F32, tag="sqzf")
            nc.vector.tensor_mul(sqzf[:, :], znf[:, :], znf[:, :])
            nc.vector.tensor_reduce(
                ssqf_sb[:, :], sqzf[:, :].rearrange("p (i c) -> p i c", i=NIC),
                mybir.AxisListType.X, ALU.add)
            for ic in range(NIC):
                zfT_ps = PT.tile([64, 128], BF16, tag="ztrf",
                                 name="zfT_ps", bufs=2)
                nc.tensor.transpose(zfT_ps[:, :],
                                    znf[:, ic * DF:(ic + 1) * DF],
                                    ident_sb[:, :])
                nc.vector.tensor_copy(znfT_sb[:, ic * 128:(ic + 1) * 128],
                                      zfT_ps[:, :])
            # fg z out + AllGather A
            nc.sync.dma_start(zpackF.ap(), znfT_sb[:, :])
            ag_f = nc.gpsimd.collective_compute(
                "AllGather", ALU.bypass, replica_groups=rg,
                ins=[zpackF.ap().opt()], outs=[zgathF.ap().opt()])
            add_dep_helper(ag_f.ins, dum_inst.ins,
                           reason="dummy barrier-eater before AGf")
            PTctx.__exit__(None, None, None)
            PZctx.__exit__(None, None, None)

            # ================= cls head pipeline =================
            hTc_sb = P.tile([128, (H // 128) * SH], BF16, tag="hTc")
            PH2ctx = tc.tile_pool(name="ph2", bufs=1, space="PSUM")
            PH2 = PH2ctx.__enter__()
            for m in range(H // 128):
                pq = [PH2.tile([128, 256], F32, tag=f"cps{q}",
                               name=f"cps{q}", bufs=(2 if q < 3 else 1))
                      for q in range(4)]
                for c in range(C // 128):
                    for q in range(4):
                        nc.tensor.matmul(
                            pq[q][:, :],
                            lhsT=w1c_sb[:, c * H + m * 128:c * H + (m + 1) * 128],
                            rhs=xT_sb[:, c * SH + q * 256:c * SH + q * 256 + 256],
                            start=(c == 0), stop=(c == C // 128 - 1))
                for q in range(4):
                    nc.vector.tensor_scalar(
                        hTc_sb[:, m * SH + q * 256:m * SH + q * 256 + 256],
                        pq[q][:, :], b1c_sb[:, m:m + 1], 0.0,
                        ALU.add, ALU.max)
            PH2ctx.__exit__(None, None, None)

            PZ2ctx = tc.tile_pool(name="pzc", bufs=1, space="PSUM")
            PZ2 = PZ2ctx.__enter__()
            PT2ctx = tc.tile_pool(name="ptrc", bufs=1, space="PSUM")
            PT2 = PT2ctx.__enter__()
            PCctx = tc.tile_pool(name="pcb", bufs=1, space="PSUM")
            PC = PCctx.__enter__()
            zc_ps = PZ2.tile([128, NIC * DC], F32, tag="zc")     # 2 banks
            for ic in range(NIC):
                for hm in range(H // 128):
                    nc.tensor.matmul(
                        zc_ps[:, ic * DC:(ic + 1) * DC],
                        lhsT=hTc_sb[:, hm * SH + ic * 128:hm * SH + ic * 128 + 128],
                        rhs=w2c_sb[:, hm * DC:(hm + 1) * DC],
                        start=(hm == 0), stop=(hm == H // 128 - 1))
            ztc = P.tile([128, NIC * DC], F32, tag="ztc")
            nc.vector.tensor_add(ztc[:, :], zc_ps[:, :], b2c8_sb[:, :])
            sqc = W.tile([128, NIC * DC], F32, tag="sqc")
            nc.vector.tensor_mul(sqc[:, :], ztc[:, :], ztc[:, :])
            n2c = P.tile([128, NIC], F32, tag="n2c")
            nc.vector.tensor_reduce(
                n2c[:, :], sqc[:, :].rearrange("p (i c) -> p i c", i=NIC),
                mybir.AxisListType.X, ALU.add)
            lnc = P.tile([128, NIC], F32, tag="lnc")
            nc.scalar.activation(lnc[:, :], n2c[:, :], AF.Ln)
            ninvc = P.tile([128, NIC], F32, tag="ninvc")
            nc.scalar.activation(ninvc[:, :], lnc[:, :], AF.Exp, scale=-0.5)
            # normalized cls z + ones column (for the class-sum matmul)
            zcat = P.tile([128, NIC * (DC + 1)], BF16, tag="zcat")
            for ic in range(NIC):
                zoff = ic * (DC + 1)
                nc.vector.tensor_scalar_mul(
                    zcat[:, zoff:zoff + DC],
                    ztc[:, ic * DC:(ic + 1) * DC], ninvc[:, ic:ic + 1])
                nc.vector.memset(zcat[:, zoff + DC:zoff + DC + 1], 1.0)
            zc_v = zcat[:, :].rearrange("p (i c) -> p i c", i=NIC)
            sqzc = W.tile([128, NIC * DC], F32, tag="sqzc")
            sqzc_v = sqzc[:, :].rearrange("p (i c) -> p i c", i=NIC)
            nc.vector.tensor_mul(sqzc_v, zc_v[:, :, 0:DC], zc_v[:, :, 0:DC])
            nc.vector.tensor_reduce(ssqc_sb[:, :], sqzc_v,
                                    mybir.AxisListType.X, ALU.add)
            cb_ps = PC.tile([NCLS, DC + 1], F32, tag="cb")
            for ic in range(NIC):
                zoff = ic * (DC + 1)
                nc.tensor.matmul(
                    cb_ps[:, :],
                    lhsT=ohb_sb[:, ic * NCLS:(ic + 1) * NCLS],
                    rhs=zcat[:, zoff:zoff + DC + 1],
                    start=(ic == 0), stop=(ic == NIC - 1))
                zcT_ps = PT2.tile([128, 128], BF16, tag="ztrc",
                                  name="zcT_ps", bufs=2)
                nc.tensor.transpose(zcT_ps[:, :],
                                    zcat[:, zoff:zoff + DC],
                                    ident_sb[:, :])
                nc.vector.tensor_copy(zncT_sb[:, ic * 128:(ic + 1) * 128],
                                      zcT_ps[:, :])
            # cls z out + AllGather B, class sums + AllReduce
            nc.sync.dma_start(zpackC.ap(), zncT_sb[:, :])
            ag_c = nc.gpsimd.collective_compute(
                "AllGather", ALU.bypass, replica_groups=rg,
                ins=[zpackC.ap().opt()], outs=[zgathC.ap().opt()])
            add_dep_helper(ag_c.ins, ag_f.ins,
                           reason="AGf before AGc on cc stream")
            nc.vector.tensor_copy(cbl_sb[:, :], cb_ps[:, :])
            nc.sync.dma_start(cbL.ap(), cbl_sb[:, :])
            ar_inst = nc.gpsimd.collective_compute(
                "AllReduce", ALU.add, replica_groups=rg,
                ins=[cbL.ap().opt()], outs=[cbR.ap().opt()])
            add_dep_helper(ar_inst.ins, ag_c.ins,
                           reason="AGc before AR on cc stream")
            PCctx.__exit__(None, None, None)
            PT2ctx.__exit__(None, None, None)
            PZ2ctx.__exit__(None, None, None)

            # ---- cb-independent precompute (fills AG wait) ----
            edf_sb = P.tile([128, NIC], F32, tag="edf_sb")
            nc.scalar.activation(edf_sb[:, :], ssqf_sb[:, :], AF.Exp,
                                 scale=1.0 / TAU)
            edc_sb = P.tile([128, NIC], F32, tag="edc_sb")
            nc.scalar.activation(edc_sb[:, :], ssqc_sb[:, :], AF.Exp,
                                 scale=1.0 / TAU)
            t0f = P.tile([128, NIC], F32, tag="t0f")
            nc.vector.tensor_mul(t0f[:, :], edf_sb[:, :], fgown_sb[:, :])
            iouw_pre = P.tile([128, NIC], F32, tag="iouw_pre")
            thr0 = W.tile([128, NIC], F32, tag="thr0", name="thr0")
            nc.vector.tensor_scalar(thr0[:, :], iou_sb[:, :], -0.5, 1e9,
                                    ALU.add, ALU.mult)
            nc.vector.tensor_scalar_max(thr0[:, :], thr0[:, :], 0.0)
            nc.vector.tensor_scalar_min(thr0[:, :], thr0[:, :], 1.0)
            nc.vector.tensor_mul(iouw_pre[:, :], iou_sb[:, :], thr0[:, :])

            # ---- gathered z into SBUF ----
            for r in range(NCORES):
                nc.sync.dma_start(zfT_all[:, r * SH:(r + 1) * SH],
                                  zgathF.ap()[r * DF:(r + 1) * DF, :])
            for r in range(NCORES):
                nc.sync.dma_start(zcT_all[:, r * SH:(r + 1) * SH],
                                  zgathC.ap()[r * DC:(r + 1) * DC, :])

            # ================= fg j-loop =================
            with tc.tile_pool(name="psimf", bufs=2, space="PSUM") as PJ:
                for ic in range(NIC):
                    lhsT = znfT_sb[:, ic * 128:(ic + 1) * 128]
                    for jt in range(NJT):
                        sim = PJ.tile([128, JT], F32, tag="simf")
                        for q in range(JT // 512):
                            jo = jt * JT + q * 512
                            nc.tensor.matmul(
                                sim[:, q * 512:(q + 1) * 512],
                                lhsT=lhsT, rhs=zfT_all[:, jo:jo + 512],
                                start=True, stop=True)
                        nc.scalar.activation(
                            sim[:, :], sim[:, :], AF.Exp, scale=1.0 / TAU,
                            accum_out=accF[:, ic * 8 + jt:ic * 8 + jt + 1])
                    if bgw > 0:
                        simb = PJ.tile([128, JT], F32, tag="simf")
                        for q in range(bgw // 512):
                            nc.tensor.matmul(
                                simb[:, q * 512:(q + 1) * 512],
                                lhsT=lhsT,
                                rhs=zfT_all[:, q * 512:(q + 1) * 512],
                                start=True, stop=True)
                        nc.scalar.activation(
                            simb[:, 0:n_bg], simb[:, 0:n_bg], AF.Exp,
                            scale=1.0 / TAU,
                            accum_out=accF[:, ic * 8 + NJT:ic * 8 + NJT + 1])
                    else:
                        nc.vector.memset(
                            accF[:, ic * 8 + NJT:ic * 8 + NJT + 1], 0.0)

            # ---- phase 4: zbar / hist prep + spos/npos (needs cbR) ----
            with tc.tile_pool(name="p4", bufs=1, space="PSUM") as P4:
                nc.sync.dma_start(cb_sb[:, :], cbR.ap())
                zbcT_ps = P4.tile([128, NCLS], F32, tag="ps4", name="zbcT_ps",
                                  bufs=2)
                nc.tensor.transpose(zbcT_ps[:, :], cb_sb[:, 0:DC],
                                    identF_sb[0:NCLS, 0:NCLS])
                nc.vector.tensor_copy(zbcT_sb[:, :], zbcT_ps[:, :])
                hist_ps = P4.tile([1, NCLS], F32, tag="ps4", name="hist_ps",
                                  bufs=2)
                nc.tensor.transpose(hist_ps[:, :], cb_sb[:, DC:DC + 1],
                                    identF_sb[0:NCLS, 0:NCLS])
                nc.vector.tensor_copy(hist_sb[:, :], hist_ps[:, :])
                nc.vector.tensor_reduce(fgtot_sb[:, :], hist_sb[:, :],
                                        mybir.AxisListType.X, ALU.add)
                hb_ps = P4.tile([128, NCLS + 1], F32, tag="ps4", name="hb_ps",
                                bufs=2)
                nc.tensor.matmul(hb_ps[:, 0:NCLS], lhsT=onesR_sb[:, :],
                                 rhs=hist_sb[:, :], start=True, stop=True)
                nc.tensor.matmul(hb_ps[:, NCLS:NCLS + 1], lhsT=onesR_sb[:, :],
                                 rhs=fgtot_sb[:, :], start=True, stop=True)
                nc.vector.tensor_copy(histB_sb[:, :], hb_ps[:, 0:NCLS])
                nc.vector.tensor_copy(ftB_sb[:, :], hb_ps[:, NCLS:NCLS + 1])

                gall_ps = P4.tile([128, NIC * 32], F32, tag="gall")
                for ic in range(NIC):
                    nc.tensor.matmul(gall_ps[:, ic * 32:ic * 32 + NCLS],
                                     lhsT=zncT_sb[:, ic * 128:(ic + 1) * 128],
                                     rhs=zbcT_sb[:, :], start=True, stop=True)
                g_v = gall_ps[:, :].rearrange("p (i c) -> p i c", i=NIC)
                oh_v = ohb_sb[:, :].rearrange("p (i c) -> p i c", i=NIC)
                gm = W.tile([128, NIC * NCLS], F32, tag="gm")
                gm_v = gm[:, :].rearrange("p (i c) -> p i c", i=NIC)
                nc.vector.tensor_mul(gm_v, g_v[:, :, 0:NCLS], oh_v)
                nc.vector.tensor_reduce(spos_sb[:, :], gm_v,
                                        mybir.AxisListType.X, ALU.add)
                hb8 = W.tile([128, NIC * NCLS], F32, tag="hb8")
                for r in range(NIC):
                    nc.vector.tensor_copy(hb8[:, r * NCLS:(r + 1) * NCLS],
                                          histB_sb[:, :])
                nm = W.tile([128, NIC * NCLS], F32, tag="nm")
                nm_v = nm[:, :].rearrange("p (i c) -> p i c", i=NIC)
                nc.vector.tensor_mul(
                    nm_v, hb8[:, :].rearrange("p (i c) -> p i c", i=NIC), oh_v)
                nc.vector.tensor_reduce(npos_sb[:, :], nm_v,
                                        mybir.AxisListType.X, ALU.add)

                # accum-independent final-phase terms
                nposf = W.tile([128, NIC], F32, tag="nposf", name="nposf")
                nc.vector.tensor_scalar(nposf[:, :], fgown_sb[:, :], -1.0,
                                        ftB_sb[:, 0:1], ALU.mult, ALU.add)
                vf = W.tile([128, NIC], F32, tag="vf", name="vf")
                nc.vector.tensor_scalar_min(vf[:, :], nposf[:, :], 1.0)
                validf = W.tile([128, NIC], F32, tag="validf", name="validf")
                nc.vector.tensor_mul(validf[:, :], vf[:, :], fgown_sb[:, :])
                FIN = P.tile([128, 32], F32, tag="FIN")
                nc.vector.tensor_mul(FIN[:, 8:16], iouw_pre[:, :],
                                     validf[:, :])
                vc = W.tile([128, NIC], F32, tag="vc", name="vc")
                nc.vector.tensor_scalar_min(vc[:, :], npos_sb[:, :], 1.0)
                validc = W.tile([128, NIC], F32, tag="validc", name="validc")
                nc.vector.tensor_mul(validc[:, :], vc[:, :], fgown_sb[:, :])
                nc.vector.tensor_mul(FIN[:, 24:32], iouw_pre[:, :],
                                     validc[:, :])
                t2m = P.tile([128, NIC], F32, tag="t2m")
                nc.vector.tensor_sub(t2m[:, :], spos_sb[:, :], ssqc_sb[:, :])
                nc.vector.tensor_scalar(t2m[:, :], t2m[:, :], -1.0 / TAU, 1e9,
                                        ALU.mult, ALU.add)
                npm1 = P.tile([128, NIC], F32, tag="npm1s")
                nc.vector.tensor_scalar_add(npm1[:, :], npos_sb[:, :], -1.0)
                hh = W.tile([128, NIC], F32, tag="hh", name="hh")
                nc.vector.tensor_scalar_add(hh[:, :], npos_sb[:, :], EPS)
                rcp_sb = P.tile([128, NIC], F32, tag="rcp_sb")
                nc.vector.reciprocal(rcp_sb[:, :], hh[:, :])

            # ================= cls j-loop =================
            with tc.tile_pool(name="psimc", bufs=2, space="PSUM") as PJ2:
                for ic in range(NIC):
                    lhsT = zncT_sb[:, ic * 128:(ic + 1) * 128]
                    for jt in range(NJT):
                        sim = PJ2.tile([128, JT], F32, tag="simc")
                        for q in range(JT // 512):
                            jo = jt * JT + q * 512
                            nc.tensor.matmul(
                                sim[:, q * 512:(q + 1) * 512],
                                lhsT=lhsT, rhs=zcT_all[:, jo:jo + 512],
                                start=True, stop=True)
                        nc.scalar.activation(
                            sim[:, :], sim[:, :], AF.Exp, scale=1.0 / TAU,
                            accum_out=accC[:, ic * 8 + jt:ic * 8 + jt + 1])

            # ================= final assembly =================
            with tc.tile_pool(name="pfin", bufs=2, space="PSUM") as PF:
                accF_v = accF[:, :].rearrange("p (i s) -> p i s", i=NIC)
                accC_v = accC[:, :].rearrange("p (i s) -> p i s", i=NIC)
                fgdn = P.tile([128, NIC], F32, tag="fgdn")
                nc.vector.tensor_reduce(fgdn[:, :], accF_v[:, :, 0:NJT],
                                        mybir.AxisListType.X, ALU.add)
                clsdn = P.tile([128, NIC], F32, tag="clsdn")
                nc.vector.tensor_reduce(clsdn[:, :], accC_v[:, :, 0:NJT],
                                        mybir.AxisListType.X, ALU.add)
                bgs = P.tile([128, NIC], F32, tag="bgs")
                nc.vector.tensor_copy(
                    bgs[:, :].rearrange("p (i o) -> p i o", i=NIC),
                    accF_v[:, :, NJT:NJT + 1])

                def T(tag):
                    return W.tile([128, NIC], F32, tag=tag, name=tag)

                denom = T("denom")
                nc.vector.tensor_sub(denom[:, :], fgdn[:, :], edf_sb[:, :])
                numer = T("numer")
                nc.vector.tensor_sub(numer[:, :], fgdn[:, :], bgs[:, :])
                nc.vector.tensor_sub(numer[:, :], numer[:, :], t0f[:, :])
                denc = T("denc")
                nc.vector.tensor_sub(denc[:, :], clsdn[:, :], edc_sb[:, :])
                lnd = T("lnd")
                nc.scalar.activation(lnd[:, :], denom[:, :], AF.Ln,
                                     bias=eps2_sb[:, 0:1])
                lnn = T("lnn")
                nc.scalar.activation(lnn[:, :], numer[:, :], AF.Ln,
                                     bias=eps1_sb[:, 0:1])
                lndc = T("lndc")
                nc.scalar.activation(lndc[:, :], denc[:, :], AF.Ln)
                lossf = T("lossf")
                nc.vector.tensor_sub(lossf[:, :], lnd[:, :], lnn[:, :])
                nc.vector.tensor_mul(FIN[:, 0:8], FIN[:, 8:16], lossf[:, :])
                t3 = T("t3")
                nc.vector.tensor_mul(t3[:, :], npm1[:, :], lndc[:, :])
                g = T("g")
                nc.vector.tensor_add(g[:, :], t2m[:, :], t3[:, :])
                lzi = T("lzi")
                nc.vector.tensor_mul(lzi[:, :], g[:, :], rcp_sb[:, :])
                nc.vector.tensor_mul(FIN[:, 16:24], FIN[:, 24:32], lzi[:, :])

                fin_ps = PF.tile([1, 32], F32, tag="fin")
                nc.tensor.matmul(fin_ps[:, :], lhsT=onesP_sb[:, :],
                                 rhs=FIN[:, :], start=True, stop=True)
                res4 = P.tile([1, 8], F32, tag="res4")
                nc.vector.tensor_reduce(
                    res4[:, 0:4],
                    fin_ps[:, :].rearrange("p (q c) -> p q c", q=4),
                    mybir.AxisListType.X, ALU.add)
                nc.vector.tensor_copy(res4[:, 4:5], fgtot_sb[:, :])
                nc.vector.memset(res4[:, 5:8], 0.0)
                nc.sync.dma_start(psums.ap(), res4[:, :])

    nc.compile()
    return nc


def _prep_inputs(roi_feats, labels, ious, fg_w1, fg_b1, fg_w2, fg_b2,
                 cls_w1, cls_b1, cls_w2, cls_b2):
    bf = ml_dtypes.bfloat16
    labels = np.asarray(labels).astype(np.int64)
    ious = np.asarray(ious, np.float32)
    roi = np.asarray(roi_feats, np.float32)

    # permute rows: background (label==0) first; loss is invariant
    perm = np.argsort(labels != 0, kind="stable")
    n_bg = int(np.sum(labels == 0))
    roi = roi[perm]
    labels = labels[perm]
    ious = ious[perm]

    b1f = np.ascontiguousarray(
        np.asarray(fg_b1, np.float32).reshape(H // 128, 128).T)
    b1c = np.ascontiguousarray(
        np.asarray(cls_b1, np.float32).reshape(H // 128, 128).T)
    b2f8 = np.tile(np.tile(np.asarray(fg_b2, np.float32), (128, 1)),
                   (1, NIC))
    b2c8 = np.tile(np.tile(np.asarray(cls_b2, np.float32), (128, 1)),
                   (1, NIC))

    fg_glob = (labels > 0).astype(np.float32)
    ident = np.eye(128, dtype=np.float32)

    oh_glob = np.zeros((N, NCLS), np.float32)
    oh_glob[np.arange(N), labels % NCLS] = (labels > 0)

    in_maps = []
    for k in range(NCORES):
        sl = slice(k * SH, (k + 1) * SH)
        oh_own = oh_glob[sl]
        ohb = np.concatenate(
            [oh_own[ic * 128:(ic + 1) * 128] for ic in range(NIC)],
            axis=1).astype(bf)
        in_maps.append({
            "xT": np.ascontiguousarray(roi[sl].T).astype(bf),
            "w1f": np.asarray(fg_w1).astype(bf),
            "w1c": np.asarray(cls_w1).astype(bf),
            "b1f": b1f,
            "b1c": b1c,
            "w2f": np.asarray(fg_w2).astype(bf),
            "w2c": np.asarray(cls_w2).astype(bf),
            "b2f8": b2f8,
            "b2c8": b2c8,
            "fgown": np.ascontiguousarray(
                fg_glob[sl].reshape(NIC, 128).T).astype(np.float32),
            "iou": np.ascontiguousarray(
                ious[sl].reshape(NIC, 128).T).astype(np.float32),
            "ohb": ohb,
            "ident": ident.astype(bf),
            "identF": ident,
        })
    return in_maps, n_bg


def _get_nc(n_bg):
    key = ("nc", n_bg)
    if key not in _cached:
        _cached[key] = _build(n_bg)
    return _cached[key]


def run(inputs, trace=False, tmpdir=None):
    in_maps, n_bg = _prep_inputs(**inputs)
    nc = _get_nc(n_bg)
    res = bass_utils.run_bass_kernel_spmd(
        nc, in_maps, core_ids=list(range(NCORES)), trace=trace, tmpdir=tmpdir)
    swl_f = sw_f = swl_c = sw_c = 0.0
    for r in res.results:
        p = r["psums"][0].astype(np.float64)
        swl_f += p[0]; sw_f += p[1]; swl_c += p[2]; sw_c += p[3]
    loss_fg = swl_f / (sw_f + EPS)
    loss_c = swl_c / (sw_c + EPS)
    out = np.array([loss_fg, loss_c], np.float32)
    return out, res


def kernel(**inputs) -> np.ndarray:
    out, _ = run(inputs)
    return out
